# revision 23
# baseline (speedup 1.0000x reference)
"""Trainium2 Bass kernel: ExponentialMovingAverage with unbiased correction.

Reference computation (per row, independently over batch b and channel c):
    ema[t] = (1-m) * ema[t-1] + m * x[t],   ema[-1] = 0,   m = 0.01
    y[t]   = ema[t] / (1 - (1-m)^(t+1))

Strategy: the (32, 256) batch/channel dims are data-parallel -> flatten to
8192 rows of length T=8192 and shard 1024 rows to each of the 8 NeuronCores.
On a core, rows map to SBUF partitions (8 tiles of [128, 8192]); the
recurrence runs along the free axis with the DVE tensor_tensor_scan
instruction:

    state = decay[t] * state + x[t]        (op0=mult, op1=add, state fp32)

which yields u[t] = ema[t] / m exactly (scan of raw x with decay 1-m). A
final tensor_tensor multiply by the precomputed row
mc[t] = m / (1 - (1-m)^(t+1)), broadcast across all 128 partitions, produces
the corrected output in place. All compute is on VectorE; DMA in/out of the
4 MiB row-tiles triple-buffers against it.
"""

import numpy as np

import concourse.bacc as bacc
import concourse.bass as bass
import concourse.mybir as mybir
import concourse.tile as tile
from concourse._compat import get_trn_type
from concourse.bass_utils import run_bass_kernel_spmd

MOMENTUM = 0.01
B, C, T = 32, 256, 8192
N_CORES = 8
ROWS = B * C
ROWS_PER_CORE = ROWS // N_CORES  # 1024
P = 128
F_SCAN = 2048  # scan chunk along the free axis (decay tile sized to this)
# mc[t] = m / (1 - (1-m)^(t+1)) rounds to exactly m (fp32) for t > 1743, so
# only the first HEAD columns need the per-element tensor_tensor multiply on
# VectorE; the tail is a constant-scale multiply on the otherwise-idle
# ScalarE (dedicated SBUF ports — no contention with the scans).
HEAD = 2048

FP32 = mybir.dt.float32


def _mc_row() -> np.ndarray:
    """m * bias-correction row, shape [1, HEAD] fp32."""
    t = np.arange(1, HEAD + 1, dtype=np.float64)
    mc = MOMENTUM / (1.0 - (1.0 - MOMENTUM) ** t)
    return mc.astype(np.float32).reshape(1, HEAD)


def build(rows_per_core: int = ROWS_PER_CORE):
    """Build the per-core Bass program (SPMD; every core runs this)."""
    assert rows_per_core % P == 0
    n_tiles = rows_per_core // P

    nc = bacc.Bacc(
        get_trn_type() or "TRN2",
        target_bir_lowering=False,
        debug=False,
        num_devices=N_CORES,
    )
    x_d = nc.dram_tensor("x", [rows_per_core, T], FP32, kind="ExternalInput")
    mc_d = nc.dram_tensor("mc", [1, HEAD], FP32, kind="ExternalInput")
    y_d = nc.dram_tensor("y", [rows_per_core, T], FP32, kind="ExternalOutput")

    with tile.TileContext(nc) as tc:
        with (
            tc.tile_pool(name="const", bufs=1) as cpool,
            tc.tile_pool(name="psum", bufs=1, space="PSUM") as ppool,
            tc.tile_pool(name="work", bufs=5) as wpool,
        ):
            # Broadcast the correction row to all 128 partitions with a
            # stride-0 source AP (128 descriptors reading the same 8 KiB).
            # (issued on the ACT HWDGE ring so the SP ring starts on x at once)
            mc_t = cpool.tile([P, HEAD], FP32)
            mc_src = mc_d[:]
            nc.scalar.dma_start(
                mc_t[:], bass.AP(mc_src.tensor, mc_src.offset, [[0, P], [1, HEAD]])
            )

            # decay lives in PSUM: the scan then reads data0 through the
            # dedicated PSUM port instead of the shared DVE/GpSimd SBUF
            # port, so GpSimd tensor ops (the head multiplies) can stream
            # concurrently with the scans instead of lock-serializing.
            decay = ppool.tile([P, F_SCAN], FP32)
            nc.vector.memset(decay[:], 1.0 - MOMENTUM)

            assert HEAD == F_SCAN

            def spans_for_tile(i):
                """Scan/mul/out spans: uniform F_SCAN chunks."""
                return [
                    (j * F_SCAN, (j + 1) * F_SCAN) for j in range(T // F_SCAN)
                ]

            for i in range(n_tiles):
                rows = slice(i * P, (i + 1) * P)
                xt = wpool.tile([P, T], FP32)
                # u[t] = (1-m)*u[t-1] + x[t], chained across spans. Input
                # DMA, scan, correction multiply, and output DMA are all
                # span-granular so every stage streams: a span's scan
                # starts as soon as its slice lands, and its corrected
                # output leaves while the next span is still scanning.
                # Spans inside [0, HEAD) need the per-element mc row —
                # done on GpSimd so VectorE stays scan-only (the critical
                # path); later spans are a constant-scale multiply on
                # ScalarE. Outputs ride the ACT HWDGE ring
                # (qActDynamicHW); inputs the SP ring — a single shared
                # FIFO would serialize the two streams.
                spans = spans_for_tile(i)

                def mul_and_out(lo, hi):
                    if hi <= HEAD:
                        nc.gpsimd.tensor_mul(
                            xt[:, lo:hi], xt[:, lo:hi], mc_t[:, lo:hi]
                        )
                    else:
                        nc.scalar.mul(xt[:, lo:hi], xt[:, lo:hi], MOMENTUM)
                    nc.scalar.dma_start(y_d[rows, lo:hi], xt[:, lo:hi])

                for k, (lo, hi) in enumerate(spans):
                    nc.sync.dma_start(xt[:, lo:hi], x_d[rows, lo:hi])
                    nc.vector.tensor_tensor_scan(
                        xt[:, lo:hi],
                        decay[:, : hi - lo],
                        xt[:, lo:hi],
                        0.0 if lo == 0 else xt[:, lo - 1 : lo],
                        mybir.AluOpType.mult,
                        mybir.AluOpType.add,
                    )
                    # The multiply of span k-1 is emitted only now: it
                    # scales xt in place, and this span's scan needed the
                    # unscaled boundary element xt[:, lo-1] as its initial
                    # state.
                    if k > 0:
                        mul_and_out(*spans[k - 1])
                mul_and_out(*spans[-1])

    nc.finalize()  # Bacc register allocation; run_bass_kernel_spmd skips it
    return nc


_NC_CACHE = None


def _get_nc():
    global _NC_CACHE
    if _NC_CACHE is None:
        _NC_CACHE = build()
    return _NC_CACHE


def run(x: np.ndarray, trace: bool = False, trace_kwargs: dict | None = None):
    """Run on 8 NeuronCores; returns (y, BassKernelResults)."""
    x = np.asarray(x)
    assert x.shape == (B, C, T) and x.dtype == np.float32
    xr = x.reshape(ROWS, T)
    mc = _mc_row()
    in_maps = [
        {
            "x": np.ascontiguousarray(
                xr[i * ROWS_PER_CORE : (i + 1) * ROWS_PER_CORE]
            ),
            "mc": mc,
        }
        for i in range(N_CORES)
    ]
    res = run_bass_kernel_spmd(
        _get_nc(),
        in_maps,
        list(range(N_CORES)),
        trace=trace,
        **(trace_kwargs or {}),
    )
    y = np.concatenate([r["y"] for r in res.results], axis=0).reshape(B, C, T)
    return y, res


def kernel(x: np.ndarray) -> np.ndarray:
    y, _ = run(x)
    return y


# revision 24
# speedup vs baseline: 1.1577x; 1.1577x over previous
"""Trainium2 Bass kernel: ExponentialMovingAverage with unbiased correction.

Reference computation (per row, independently over batch b and channel c):
    ema[t] = (1-m) * ema[t-1] + m * x[t],   ema[-1] = 0,   m = 0.01
    y[t]   = ema[t] / (1 - (1-m)^(t+1))

Strategy: the (32, 256) batch/channel dims are data-parallel -> flatten to
8192 rows of length T=8192 and shard 1024 rows to each of the 8 NeuronCores.
On a core, rows map to SBUF partitions (8 tiles of [128, 8192]); the
recurrence runs along the free axis with the DVE tensor_tensor_scan
instruction:

    state = decay[t] * state + x[t]        (op0=mult, op1=add, state fp32)

which yields u[t] = ema[t] / m (scan of raw x with decay 1-m, linearity), so
the correction multiply y = u * mc with mc[t] = m / (1 - (1-m)^(t+1)) folds
the m back in.

Engine budget per core (~180 us e2e, at the NC-pair HBM roofline):
  - VectorE is the critical path and runs ONLY the 32 scans (~143 us; the
    scan recurrence is 2 cycles/element and no other engine may run it).
    decay sits in PSUM so the scans never touch the shared DVE/GpSimd SBUF
    port.
  - GpSimd multiplies the head spans (t < 2048) by the per-element mc row
    (broadcast to 128 partitions once, via a stride-0-source DMA).
  - ScalarE multiplies the tail spans (t >= 2048, where mc[t] == m exactly
    in fp32) by the constant m, and issues the output DMAs on its own
    HWDGE ring so in- and out-streams never share a FIFO.
  - All stages are 2048-column span-granular and 5-deep buffered, so DMA
    in, scan, multiply, and DMA out stream concurrently.
"""

import numpy as np

import concourse.bacc as bacc
import concourse.bass as bass
import concourse.mybir as mybir
import concourse.tile as tile
from concourse._compat import get_trn_type
from concourse.bass_utils import run_bass_kernel_spmd

MOMENTUM = 0.01
B, C, T = 32, 256, 8192
N_CORES = 8
ROWS = B * C
ROWS_PER_CORE = ROWS // N_CORES  # 1024
P = 128
F_SCAN = 2048  # scan chunk along the free axis (decay tile sized to this)
# mc[t] = m / (1 - (1-m)^(t+1)) rounds to exactly m (fp32) for t > 1743, so
# only the first HEAD columns need the per-element tensor_tensor multiply on
# VectorE; the tail is a constant-scale multiply on the otherwise-idle
# ScalarE (dedicated SBUF ports — no contention with the scans).
HEAD = 2048

FP32 = mybir.dt.float32


def _mc_row() -> np.ndarray:
    """m * bias-correction row, shape [1, HEAD] fp32."""
    t = np.arange(1, HEAD + 1, dtype=np.float64)
    mc = MOMENTUM / (1.0 - (1.0 - MOMENTUM) ** t)
    return mc.astype(np.float32).reshape(1, HEAD)


def build(rows_per_core: int = ROWS_PER_CORE):
    """Build the per-core Bass program (SPMD; every core runs this)."""
    assert rows_per_core % P == 0
    n_tiles = rows_per_core // P

    nc = bacc.Bacc(
        get_trn_type() or "TRN2",
        target_bir_lowering=False,
        debug=False,
        num_devices=N_CORES,
    )
    x_d = nc.dram_tensor("x", [rows_per_core, T], FP32, kind="ExternalInput")
    mc_d = nc.dram_tensor("mc", [1, HEAD], FP32, kind="ExternalInput")
    y_d = nc.dram_tensor("y", [rows_per_core, T], FP32, kind="ExternalOutput")

    with tile.TileContext(nc) as tc:
        with (
            tc.tile_pool(name="const", bufs=1) as cpool,
            tc.tile_pool(name="psum", bufs=1, space="PSUM") as ppool,
            tc.tile_pool(name="work", bufs=5) as wpool,
        ):
            # Broadcast the correction row to all 128 partitions with a
            # stride-0 source AP (128 descriptors reading the same 8 KiB).
            # (issued on the ACT HWDGE ring so the SP ring starts on x at once)
            mc_t = cpool.tile([P, HEAD], FP32)
            mc_src = mc_d[:]
            nc.scalar.dma_start(
                mc_t[:], bass.AP(mc_src.tensor, mc_src.offset, [[0, P], [1, HEAD]])
            )

            # decay lives in PSUM: the scan then reads data0 through the
            # dedicated PSUM port instead of the shared DVE/GpSimd SBUF
            # port, so GpSimd tensor ops (the head multiplies) can stream
            # concurrently with the scans instead of lock-serializing.
            decay = ppool.tile([P, F_SCAN], FP32)
            nc.vector.memset(decay[:], 1.0 - MOMENTUM)

            assert HEAD == F_SCAN

            def spans_for_tile(i):
                """Scan/mul/out spans: uniform F_SCAN chunks."""
                return [
                    (j * F_SCAN, (j + 1) * F_SCAN) for j in range(T // F_SCAN)
                ]

            for i in range(n_tiles):
                rows = slice(i * P, (i + 1) * P)
                xt = wpool.tile([P, T], FP32)
                # u[t] = (1-m)*u[t-1] + x[t], chained across spans. Input
                # DMA, scan, correction multiply, and output DMA are all
                # span-granular so every stage streams: a span's scan
                # starts as soon as its slice lands, and its corrected
                # output leaves while the next span is still scanning.
                # Spans inside [0, HEAD) need the per-element mc row —
                # done on GpSimd so VectorE stays scan-only (the critical
                # path); later spans are a constant-scale multiply on
                # ScalarE. Outputs ride the ACT HWDGE ring
                # (qActDynamicHW); inputs the SP ring — a single shared
                # FIFO would serialize the two streams.
                spans = spans_for_tile(i)

                def mul_and_out(lo, hi):
                    if hi <= HEAD:
                        nc.gpsimd.tensor_mul(
                            xt[:, lo:hi], xt[:, lo:hi], mc_t[:, lo:hi]
                        )
                    else:
                        nc.scalar.mul(xt[:, lo:hi], xt[:, lo:hi], MOMENTUM)
                    nc.scalar.dma_start(y_d[rows, lo:hi], xt[:, lo:hi])

                for k, (lo, hi) in enumerate(spans):
                    nc.sync.dma_start(xt[:, lo:hi], x_d[rows, lo:hi])
                    nc.vector.tensor_tensor_scan(
                        xt[:, lo:hi],
                        decay[:, : hi - lo],
                        xt[:, lo:hi],
                        0.0 if lo == 0 else xt[:, lo - 1 : lo],
                        mybir.AluOpType.mult,
                        mybir.AluOpType.add,
                    )
                    # The multiply of span k-1 is emitted only now: it
                    # scales xt in place, and this span's scan needed the
                    # unscaled boundary element xt[:, lo-1] as its initial
                    # state.
                    if k > 0:
                        mul_and_out(*spans[k - 1])
                mul_and_out(*spans[-1])

    nc.finalize()  # Bacc register allocation; run_bass_kernel_spmd skips it
    return nc


_NC_CACHE = None


def _get_nc():
    global _NC_CACHE
    if _NC_CACHE is None:
        _NC_CACHE = build()
    return _NC_CACHE


def run(x: np.ndarray, trace: bool = False, trace_kwargs: dict | None = None):
    """Run on 8 NeuronCores; returns (y, BassKernelResults)."""
    x = np.asarray(x)
    assert x.shape == (B, C, T) and x.dtype == np.float32
    xr = x.reshape(ROWS, T)
    mc = _mc_row()
    in_maps = [
        {
            "x": np.ascontiguousarray(
                xr[i * ROWS_PER_CORE : (i + 1) * ROWS_PER_CORE]
            ),
            "mc": mc,
        }
        for i in range(N_CORES)
    ]
    res = run_bass_kernel_spmd(
        _get_nc(),
        in_maps,
        list(range(N_CORES)),
        trace=trace,
        **(trace_kwargs or {}),
    )
    y = np.concatenate([r["y"] for r in res.results], axis=0).reshape(B, C, T)
    return y, res


def kernel(x: np.ndarray) -> np.ndarray:
    y, _ = run(x)
    return y


# revision 26
# speedup vs baseline: 1.1652x; 1.0065x over previous
"""Trainium2 Bass kernel: ExponentialMovingAverage with unbiased correction.

Reference computation (per row, independently over batch b and channel c):
    ema[t] = (1-m) * ema[t-1] + m * x[t],   ema[-1] = 0,   m = 0.01
    y[t]   = ema[t] / (1 - (1-m)^(t+1))

Strategy: the (32, 256) batch/channel dims are data-parallel -> flatten to
8192 rows of length T=8192 and shard 1024 rows to each of the 8 NeuronCores.
On a core, rows map to SBUF partitions (8 tiles of [128, 8192]); the
recurrence runs along the free axis with the DVE tensor_tensor_scan
instruction:

    state = decay[t] * state + x[t]        (op0=mult, op1=add, state fp32)

which yields u[t] = ema[t] / m (scan of raw x with decay 1-m, linearity), so
the correction multiply y = u * mc with mc[t] = m / (1 - (1-m)^(t+1)) folds
the m back in.

Engine budget per core (~180 us e2e, at the NC-pair HBM roofline):
  - VectorE is the critical path and runs ONLY the 32 scans (~143 us; the
    scan recurrence is 2 cycles/element and no other engine may run it).
    decay sits in PSUM so the scans never touch the shared DVE/GpSimd SBUF
    port.
  - GpSimd multiplies the head spans (t < 2048) by the per-element mc row
    (broadcast to 128 partitions once, via a stride-0-source DMA).
  - ScalarE multiplies the tail spans (t >= 2048, where mc[t] == m exactly
    in fp32) by the constant m, and issues the output DMAs on its own
    HWDGE ring so in- and out-streams never share a FIFO.
  - All stages are 2048-column span-granular and 5-deep buffered, so DMA
    in, scan, multiply, and DMA out stream concurrently.
"""

import numpy as np

import concourse.bacc as bacc
import concourse.bass as bass
import concourse.mybir as mybir
import concourse.tile as tile
from concourse._compat import get_trn_type
from concourse.bass_utils import run_bass_kernel_spmd

MOMENTUM = 0.01
B, C, T = 32, 256, 8192
N_CORES = 8
ROWS = B * C
ROWS_PER_CORE = ROWS // N_CORES  # 1024
P = 128
F_SCAN = 2048  # scan chunk along the free axis (decay tile sized to this)
# mc[t] = m / (1 - (1-m)^(t+1)) rounds to exactly m (fp32) for t > 1743, so
# only the first HEAD columns need the per-element tensor_tensor multiply on
# VectorE; the tail is a constant-scale multiply on the otherwise-idle
# ScalarE (dedicated SBUF ports — no contention with the scans).
HEAD = 2048

FP32 = mybir.dt.float32


def _mc_row() -> np.ndarray:
    """m * bias-correction row, shape [1, HEAD] fp32."""
    t = np.arange(1, HEAD + 1, dtype=np.float64)
    mc = MOMENTUM / (1.0 - (1.0 - MOMENTUM) ** t)
    return mc.astype(np.float32).reshape(1, HEAD)


def build(rows_per_core: int = ROWS_PER_CORE):
    """Build the per-core Bass program (SPMD; every core runs this)."""
    assert rows_per_core % P == 0
    n_tiles = rows_per_core // P

    nc = bacc.Bacc(
        get_trn_type() or "TRN2",
        target_bir_lowering=False,
        debug=False,
        num_devices=N_CORES,
    )
    x_d = nc.dram_tensor("x", [rows_per_core, T], FP32, kind="ExternalInput")
    mc_d = nc.dram_tensor("mc", [1, HEAD], FP32, kind="ExternalInput")
    y_d = nc.dram_tensor("y", [rows_per_core, T], FP32, kind="ExternalOutput")

    with tile.TileContext(nc) as tc:
        with (
            tc.tile_pool(name="const", bufs=1) as cpool,
            tc.tile_pool(name="psum", bufs=1, space="PSUM") as ppool,
            tc.tile_pool(name="work", bufs=5) as wpool,
        ):
            # Broadcast the correction row to all 128 partitions with a
            # stride-0 source AP (128 descriptors reading the same 8 KiB).
            # (issued on the ACT HWDGE ring so the SP ring starts on x at once)
            mc_t = cpool.tile([P, HEAD], FP32)
            mc_src = mc_d[:]
            nc.scalar.dma_start(
                mc_t[:], bass.AP(mc_src.tensor, mc_src.offset, [[0, P], [1, HEAD]])
            )

            # decay lives in PSUM: the scan then reads data0 through the
            # dedicated PSUM port instead of the shared DVE/GpSimd SBUF
            # port, so GpSimd tensor ops (the head multiplies) can stream
            # concurrently with the scans instead of lock-serializing.
            # [P, 4096] fp32 = 16 KiB/partition = exactly all 8 PSUM banks.
            decay = ppool.tile([P, 2 * F_SCAN], FP32)
            nc.vector.memset(decay[:], 1.0 - MOMENTUM)

            assert HEAD == F_SCAN

            def scan_spans_for_tile(i):
                """Scan (and input-DMA) spans. Middle tiles use 4096-wide
                scans (halves per-instruction overhead on the critical
                VectorE stream). The kernel's very first chunk is split
                fine so the first scan starts ~4us earlier (pipeline
                fill), and the last tile's tail is split fine so the last
                corrected output leaves ~4us earlier (drain)."""
                if i == 0:
                    return [
                        (0, 1024),
                        (1024, 2048),
                        (2048, 4096),
                        (4096, 8192),
                    ]
                if i == n_tiles - 1:
                    return [(0, 4096), (4096, 6144)] + [
                        (lo, lo + 512) for lo in range(6144, 8192, 512)
                    ]
                return [(0, 4096), (4096, 8192)]

            def mul_spans_for_tile(i):
                """Correction-multiply / output-DMA spans: F_SCAN chunks
                (the head/tail boundary sits at HEAD == F_SCAN), refined at
                the kernel's drain edge to match the fine tail scans."""
                if i == n_tiles - 1:
                    return [(0, 2048), (2048, 4096), (4096, 6144)] + [
                        (lo, lo + 512) for lo in range(6144, 8192, 512)
                    ]
                return [
                    (j * F_SCAN, (j + 1) * F_SCAN) for j in range(T // F_SCAN)
                ]

            for i in range(n_tiles):
                rows = slice(i * P, (i + 1) * P)
                xt = wpool.tile([P, T], FP32)
                # u[t] = (1-m)*u[t-1] + x[t], chained across spans. Input
                # DMA, scan, correction multiply, and output DMA are all
                # span-granular so every stage streams: a span's scan
                # starts as soon as its slice lands, and its corrected
                # output leaves while the next span is still scanning.
                # Spans inside [0, HEAD) need the per-element mc row —
                # done on GpSimd so VectorE stays scan-only (the critical
                # path); later spans are a constant-scale multiply on
                # ScalarE. Outputs ride the ACT HWDGE ring
                # (qActDynamicHW); inputs the SP ring — a single shared
                # FIFO would serialize the two streams.
                scan_spans = scan_spans_for_tile(i)

                def mul_and_out(lo, hi):
                    if hi <= HEAD:
                        nc.gpsimd.tensor_mul(
                            xt[:, lo:hi], xt[:, lo:hi], mc_t[:, lo:hi]
                        )
                    else:
                        nc.scalar.mul(xt[:, lo:hi], xt[:, lo:hi], MOMENTUM)
                    nc.scalar.dma_start(y_d[rows, lo:hi], xt[:, lo:hi])

                # A multiply scales xt in place, so it may only be emitted
                # once (a) its span is fully scanned and (b) every later
                # scan that reads a boundary element xt[:, lo-1] inside the
                # span (as its initial state, unscaled) has been emitted.
                pending = mul_spans_for_tile(i)
                for k, (lo, hi) in enumerate(scan_spans):
                    nc.sync.dma_start(xt[:, lo:hi], x_d[rows, lo:hi])
                    nc.vector.tensor_tensor_scan(
                        xt[:, lo:hi],
                        decay[:, : hi - lo],
                        xt[:, lo:hi],
                        0.0 if lo == 0 else xt[:, lo - 1 : lo],
                        mybir.AluOpType.mult,
                        mybir.AluOpType.add,
                    )
                    future_inits = [s[0] - 1 for s in scan_spans[k + 1 :]]
                    ready = [
                        m
                        for m in pending
                        if m[1] <= hi
                        and not any(m[0] <= t < m[1] for t in future_inits)
                    ]
                    for m in ready:
                        mul_and_out(*m)
                        pending.remove(m)
                assert not pending

    nc.finalize()  # Bacc register allocation; run_bass_kernel_spmd skips it
    return nc


_NC_CACHE = None


def _get_nc():
    global _NC_CACHE
    if _NC_CACHE is None:
        _NC_CACHE = build()
    return _NC_CACHE


def run(x: np.ndarray, trace: bool = False, trace_kwargs: dict | None = None):
    """Run on 8 NeuronCores; returns (y, BassKernelResults)."""
    x = np.asarray(x)
    assert x.shape == (B, C, T) and x.dtype == np.float32
    xr = x.reshape(ROWS, T)
    mc = _mc_row()
    in_maps = [
        {
            "x": np.ascontiguousarray(
                xr[i * ROWS_PER_CORE : (i + 1) * ROWS_PER_CORE]
            ),
            "mc": mc,
        }
        for i in range(N_CORES)
    ]
    res = run_bass_kernel_spmd(
        _get_nc(),
        in_maps,
        list(range(N_CORES)),
        trace=trace,
        **(trace_kwargs or {}),
    )
    y = np.concatenate([r["y"] for r in res.results], axis=0).reshape(B, C, T)
    return y, res


def kernel(x: np.ndarray) -> np.ndarray:
    y, _ = run(x)
    return y
